# revision 9
# baseline (speedup 1.0000x reference)
"""CrossScaleAttention Trainium2 kernel.

Full (unsharded) contract: kernel(query, key, value) with shapes
  query/key/value: (4, 4096, 256) float32  ->  out (4, 4096, 256) float32

reference math:
  q = l2norm(query); k = l2norm(key)
  out = softmax((q @ k^T) * 32**-0.5) @ value

Sharding: 8 cores; core c computes batch c//2, query rows (c%2)*2048..+2048,
with that batch's full K/V resident per core (no collectives).

Per-core algorithm (all matmuls contract over the partition dim):
  - normalize Q and K rows in natural layout, PE-transpose into
    K^T/Q^T [d=128x2, tokens] so the QK matmul can contract over d.
    SCALE is folded into Q's normalization; 1/||x|| = exp(-0.5*ln(ss)) so
    the whole kernel uses a single ACT table set (natural_log_exp).
  - S^T super-chunks [128 keys, 2, 512 queries] on PE (2 key-chunks per
    PSUM tile = 2 banks); one exp per super-chunk PSUM->SBUF producing
    P^T. No max-subtraction needed: scores are cosine sims * 0.177, so
    exp is safely bounded.
  - AV: out_psum[128 q, 258] += P^T_chunk.T @ [V | 1 1]; the ones
    columns accumulate the softmax denominator in the same matmul chain
    (padded to 258 because fp32r matmuls need an even moving-dim).
  - epilogue: out = out_psum[:, :256] * (1 / out_psum[:, 256]).
"""

import math
import sys

if "/opt/trn_rl_repo" not in sys.path:
    sys.path.insert(0, "/opt/trn_rl_repo")

import numpy as np

import concourse.bass as bass
import concourse.mybir as mybir
import concourse.tile as tile
from concourse import bacc
from concourse.bass_utils import run_bass_kernel_spmd
from concourse.masks import make_identity

F32 = mybir.dt.float32
F32R = mybir.dt.float32r
BF16 = mybir.dt.bfloat16

# dtype knobs for the two matmul stages (float32r = full-rate fp32 PE mode)
QK_DT = F32R
AV_DT = F32R

B, NQ_FULL, NK, D = 4, 4096, 4096, 256
N_CORES = 8
NQ = NQ_FULL * B // N_CORES  # 2048 queries per core
P = 128
DC = D // P          # 2 d-chunks
KC = NK // P         # 32 key chunks
SC = 2               # key chunks per super-chunk (shared exp)
KS = KC // SC        # super-chunks
QB = 512             # queries per block
NB = NQ // QB        # 4 blocks
QT = QB // P         # 4 q-subtiles per block
VW = D + 2           # V columns padded with two 1.0 columns (even moving-dim)
SCALE = float(D // 8) ** -0.5  # head_dim**-0.5 = 32**-0.5


def _build_program():
    nc = bacc.Bacc(
        "TRN2",
        target_bir_lowering=False,
        debug=False,
        enable_asserts=False,
        num_devices=N_CORES,
    )
    q_d = nc.dram_tensor("q", (NQ, D), F32, kind="ExternalInput").ap()
    k_d = nc.dram_tensor("k", (NK, D), F32, kind="ExternalInput").ap()
    v_d = nc.dram_tensor("v", (NK, D), F32, kind="ExternalInput").ap()
    o_d = nc.dram_tensor("o", (NQ, D), F32, kind="ExternalOutput").ap()

    Exp = mybir.ActivationFunctionType.Exp
    Ln = mybir.ActivationFunctionType.Ln

    with tile.TileContext(nc) as tc:
        with (
            tc.tile_pool(name="const", bufs=1) as const_pool,
            tc.tile_pool(name="persist", bufs=1) as persist,
            tc.tile_pool(name="loads", bufs=4) as loads,
            tc.tile_pool(name="small", bufs=6) as small,
            tc.tile_pool(name="pt", bufs=3) as pt_pool,
            tc.tile_pool(name="outs", bufs=3) as out_pool,
            tc.tile_pool(name="ps", bufs=2, space="PSUM") as ps_pool,
            tc.tile_pool(name="avps", bufs=1, space="PSUM") as av_pool,
        ):
            ident = const_pool.tile([P, P], F32)
            make_identity(nc, ident)

            # persistent operands
            kt = persist.tile([P, DC, NK], QK_DT)   # K^T: [d, keys]
            qt = persist.tile([P, DC, NQ], QK_DT)   # Q^T: [d, queries] (pre-scaled)
            va = persist.tile([P, KC, VW], AV_DT)   # [keys, d | ones ones]
            ones = const_pool.tile([P, 1], F32)
            nc.vector.memset(ones, 1.0)
            nc.vector.tensor_copy(
                va[:, :, D:VW], ones[:, :, None].to_broadcast((P, KC, 2))
            )

            # V loads first: no deps, fills DMA early
            for i in range(KC):
                if AV_DT == F32R or AV_DT == F32:
                    vload = loads.tile([P, D], F32, tag="nat", name=f"vload_{i}")
                    nc.sync.dma_start(vload, v_d[i * P : (i + 1) * P, :])
                    nc.vector.tensor_copy(va[:, i, :D], vload)
                else:
                    vload = loads.tile([P, D], F32, tag="nat", name=f"vload_{i}")
                    nc.sync.dma_start(vload, v_d[i * P : (i + 1) * P, :])
                    nc.vector.tensor_copy(va[:, i, :D], vload)

            def normalize_transpose(src_d, n_tiles, dst, pre_scale, label):
                """Load [128,256] natural tiles, scale rows by
                pre_scale/||row||, transpose into dst[:, dc, i*128:(i+1)*128].
                rsqrt is computed as exp(-0.5 ln + ln(pre_scale)): same ACT
                table set as the softmax exp, so no table switches."""
                if pre_scale == 1.0:
                    ln_bias = 0.0
                else:
                    bias_t = const_pool.tile(
                        [P, 1], F32, tag=f"bias{label}", name=f"bias{label}"
                    )
                    nc.vector.memset(bias_t, math.log(pre_scale))
                    ln_bias = bias_t
                for i in range(n_tiles):
                    nat = loads.tile([P, D], F32, tag="nat", name=f"{label}nat{i}")
                    nc.sync.dma_start(nat, src_d[i * P : (i + 1) * P, :])
                    sq = loads.tile([P, D], F32, tag="sq", name=f"{label}sq{i}")
                    nc.vector.tensor_mul(sq, nat, nat)
                    ss = small.tile([P, 1], F32, tag="ss", name=f"{label}ss{i}")
                    nc.vector.tensor_reduce(
                        ss, sq, axis=mybir.AxisListType.X, op=mybir.AluOpType.add
                    )
                    lg = small.tile([P, 1], F32, tag="lg", name=f"{label}lg{i}")
                    nc.scalar.activation(lg, ss, Ln)
                    rinv = small.tile([P, 1], F32, tag="rinv", name=f"{label}ri{i}")
                    nc.scalar.activation(rinv, lg, Exp, scale=-0.5, bias=ln_bias)
                    xn = loads.tile([P, D], F32, tag="xn", name=f"{label}xn{i}")
                    nc.vector.tensor_scalar_mul(xn, nat, rinv)
                    tps = ps_pool.tile([P, SC, QB], F32, tag="st", name=f"{label}tp{i}")
                    for dc in range(DC):
                        nc.tensor.transpose(
                            tps[:, 0, dc * P : (dc + 1) * P],
                            xn[:, dc * P : (dc + 1) * P],
                            ident,
                        )
                    nc.vector.tensor_copy(
                        dst[:, :, i * P : (i + 1) * P],
                        tps[:, 0, :D].rearrange("p (c n) -> p c n", c=DC),
                    )

            # Q first: K-prep then pipelines under block 0 of the main loop
            normalize_transpose(q_d, NQ // P, qt, SCALE, "q")
            normalize_transpose(k_d, KC, kt, 1.0, "k")

            # main loop
            for blk in range(NB):
                avs = [
                    av_pool.tile([P, VW], F32, tag=f"av{t}", name=f"av{t}_{blk}")
                    for t in range(QT)
                ]
                for sk in range(KS):
                    st = ps_pool.tile([P, SC, QB], F32, tag="st", name=f"st{blk}_{sk}")
                    for j in range(SC):
                        for dc in range(DC):
                            nc.tensor.matmul(
                                st[:, j, :],
                                lhsT=kt[:, dc, (sk * SC + j) * P : (sk * SC + j + 1) * P],
                                rhs=qt[:, dc, blk * QB : (blk + 1) * QB],
                                start=(dc == 0),
                                stop=(dc == DC - 1),
                            )
                    pt = pt_pool.tile([P, SC, QB], AV_DT, tag="pt", name=f"pt{blk}_{sk}")
                    nc.scalar.activation(pt, st, Exp)
                    for j in range(SC):
                        kk = sk * SC + j
                        for t in range(QT):
                            nc.tensor.matmul(
                                avs[t],
                                lhsT=pt[:, j, t * P : (t + 1) * P],
                                rhs=va[:, kk, :],
                                start=(kk == 0),
                                stop=(kk == KC - 1),
                            )
                for t in range(QT):
                    rec = small.tile([P, 1], F32, tag="rec")
                    nc.vector.reciprocal(rec, avs[t][:, D : D + 1])
                    ot = out_pool.tile([P, D], F32, tag="ot")
                    nc.vector.tensor_scalar_mul(ot, avs[t][:, :D], rec)
                    row = blk * QB + t * P
                    nc.sync.dma_start(o_d[row : row + P, :], ot)

    nc.compile()
    return nc


_CACHED = {}


def _get_program():
    if "nc" not in _CACHED:
        _CACHED["nc"] = _build_program()
    return _CACHED["nc"]


def _make_in_maps(query, key, value):
    in_maps = []
    for c in range(N_CORES):
        b = c // (N_CORES // B)
        qs = (c % (N_CORES // B)) * NQ
        in_maps.append(
            {
                "q": np.ascontiguousarray(query[b, qs : qs + NQ], dtype=np.float32),
                "k": np.ascontiguousarray(key[b], dtype=np.float32),
                "v": np.ascontiguousarray(value[b], dtype=np.float32),
            }
        )
    return in_maps


def run_sharded(query, key, value, trace=False):
    """Returns (out, BassKernelResults)."""
    nc = _get_program()
    in_maps = _make_in_maps(query, key, value)
    res = run_bass_kernel_spmd(nc, in_maps, core_ids=list(range(N_CORES)), trace=trace)
    out = np.empty((B, NQ_FULL, D), dtype=np.float32)
    for c in range(N_CORES):
        b = c // (N_CORES // B)
        qs = (c % (N_CORES // B)) * NQ
        out[b, qs : qs + NQ] = res.results[c]["o"]
    return out, res


def kernel(query, key, value):
    query = np.asarray(query)
    key = np.asarray(key)
    value = np.asarray(value)
    out, _ = run_sharded(query, key, value)
    return out


# revision 11
# speedup vs baseline: 1.3254x; 1.3254x over previous
"""CrossScaleAttention Trainium2 kernel.

Full (unsharded) contract: kernel(query, key, value) with shapes
  query/key/value: (4, 4096, 256) float32  ->  out (4, 4096, 256) float32

reference math:
  q = l2norm(query); k = l2norm(key)
  out = softmax((q @ k^T) * 32**-0.5) @ value

Sharding: 8 cores; core c computes batch c//2, query rows (c%2)*2048..+2048,
with that batch's full K/V resident per core (no collectives).

Per-core algorithm (all matmuls contract over the partition dim):
  - normalize Q and K rows in natural layout, PE-transpose into
    K^T/Q^T [d=128x2, tokens] so the QK matmul can contract over d.
    SCALE is folded into Q's normalization; 1/||x|| = exp(-0.5*ln(ss)) so
    the whole kernel uses a single ACT table set (natural_log_exp).
  - S^T super-chunks [128 keys, 2, 512 queries] on PE (2 key-chunks per
    PSUM tile = 2 banks); one exp per super-chunk PSUM->SBUF producing
    P^T. No max-subtraction needed: scores are cosine sims * 0.177, so
    exp is safely bounded.
  - AV: out_psum[128 q, 258] += P^T_chunk.T @ [V | 1 1]; the ones
    columns accumulate the softmax denominator in the same matmul chain
    (padded to 258 because fp32r matmuls need an even moving-dim).
  - epilogue: out = out_psum[:, :256] * (1 / out_psum[:, 256]).
"""

import math
import sys

if "/opt/trn_rl_repo" not in sys.path:
    sys.path.insert(0, "/opt/trn_rl_repo")

import numpy as np

import concourse.bass as bass
import concourse.mybir as mybir
import concourse.tile as tile
from concourse import bacc
from concourse.bass_utils import run_bass_kernel_spmd
from concourse.masks import make_identity

F32 = mybir.dt.float32
F32R = mybir.dt.float32r
BF16 = mybir.dt.bfloat16

# dtype knobs for the two matmul stages (float32r = full-rate fp32 PE mode)
QK_DT = F32R
AV_DT = F32R

B, NQ_FULL, NK, D = 4, 4096, 4096, 256
N_CORES = 8
NQ = NQ_FULL * B // N_CORES  # 2048 queries per core
P = 128
DC = D // P          # 2 d-chunks
KC = NK // P         # 32 key chunks
SC = 2               # key chunks per super-chunk (shared exp)
KS = KC // SC        # super-chunks
QB = 512             # queries per block
NB = NQ // QB        # 4 blocks
QT = QB // P         # 4 q-subtiles per block
VW = D + 2           # V columns padded with two 1.0 columns (even moving-dim)
SCALE = float(D // 8) ** -0.5  # head_dim**-0.5 = 32**-0.5


def _build_program():
    nc = bacc.Bacc(
        "TRN2",
        target_bir_lowering=False,
        debug=False,
        enable_asserts=False,
        num_devices=N_CORES,
    )
    q_d = nc.dram_tensor("q", (NQ, D), F32, kind="ExternalInput").ap()
    k_d = nc.dram_tensor("k", (NK, D), F32, kind="ExternalInput").ap()
    v_d = nc.dram_tensor("v", (NK, D), F32, kind="ExternalInput").ap()
    o_d = nc.dram_tensor("o", (NQ, D), F32, kind="ExternalOutput").ap()

    Exp = mybir.ActivationFunctionType.Exp
    Ln = mybir.ActivationFunctionType.Ln

    with tile.TileContext(nc) as tc:
        with (
            tc.tile_pool(name="const", bufs=1) as const_pool,
            tc.tile_pool(name="persist", bufs=1) as persist,
            tc.tile_pool(name="loads", bufs=4) as loads,
            tc.tile_pool(name="small", bufs=6) as small,
            tc.tile_pool(name="pt", bufs=3) as pt_pool,
            tc.tile_pool(name="outs", bufs=3) as out_pool,
            tc.tile_pool(name="ps", bufs=2, space="PSUM") as ps_pool,
            tc.tile_pool(name="avps", bufs=1, space="PSUM") as av_pool,
        ):
            ident = const_pool.tile([P, P], F32)
            make_identity(nc, ident)

            # persistent operands
            kt = persist.tile([P, DC, NK], QK_DT)   # K^T: [d, keys]
            qt = persist.tile([P, DC, NQ], QK_DT)   # Q^T: [d, queries] (pre-scaled)
            va = persist.tile([P, KC, VW], AV_DT)   # [keys, d | ones ones]
            ones = const_pool.tile([P, 1], F32)
            nc.vector.memset(ones, 1.0)
            nc.vector.tensor_copy(
                va[:, :, D:VW], ones[:, :, None].to_broadcast((P, KC, 2))
            )

            # V loads: no deps, fills DMA early
            for i in range(KC):
                vload = loads.tile([P, D], F32, tag="vl", name=f"vload_{i}")
                nc.sync.dma_start(vload, v_d[i * P : (i + 1) * P, :])
                nc.vector.tensor_copy(va[:, i, :D], vload)

            # ---- prologue: l2-normalize K and Q rows, transpose into kt/qt.
            # All 48 row-norm reductions accumulate into ONE [128,48] tile so
            # the rsqrt costs a single Ln + single Exp (batched by ACT table
            # set: Square lives in every set, so no table thrashing).
            QTI = NQ // P  # 16 q tiles
            NT = KC + QTI  # K tiles are idx 0..31, Q tiles idx 32..47
            natall = persist.tile([P, NT, D], F32)
            ssall = persist.tile([P, NT], F32)

            def src_ap(i):
                return (
                    k_d[i * P : (i + 1) * P, :]
                    if i < KC
                    else q_d[(i - KC) * P : (i - KC + 1) * P, :]
                )

            for i in range(NT):
                nc.sync.dma_start(natall[:, i, :], src_ap(i))
                sq = loads.tile([P, D], F32, tag="sq", name=f"sq{i}")
                nc.scalar.activation(
                    sq,
                    natall[:, i, :],
                    mybir.ActivationFunctionType.Square,
                    accum_out=ssall[:, i : i + 1],
                )
            lg = persist.tile([P, NT], F32)
            nc.scalar.activation(lg, ssall, Ln)
            rinv_all = persist.tile([P, NT], F32)
            nc.scalar.activation(rinv_all, lg, Exp, scale=-0.5)
            # fold the softmax scale into Q's normalization
            nc.vector.tensor_scalar_mul(rinv_all[:, KC:NT], rinv_all[:, KC:NT], SCALE)

            def finish_tile(i):
                """scale rows + PE-transpose into kt/qt."""
                dst, col = (kt, i) if i < KC else (qt, i - KC)
                xn = loads.tile([P, D], F32, tag="xn", name=f"xn{i}")
                nc.vector.tensor_scalar_mul(
                    xn, natall[:, i, :], rinv_all[:, i : i + 1]
                )
                tps = ps_pool.tile([P, SC, QB], F32, tag="st", name=f"tp{i}")
                for dc in range(DC):
                    nc.tensor.transpose(
                        tps[:, 0, dc * P : (dc + 1) * P],
                        xn[:, dc * P : (dc + 1) * P],
                        ident,
                    )
                nc.vector.tensor_copy(
                    dst[:, :, col * P : (col + 1) * P],
                    tps[:, 0, :D].rearrange("p (c n) -> p c n", c=DC),
                )

            # block 0 needs q-tiles 0-3 and k-tiles in order; emit those first
            for i in list(range(KC, KC + 4)) + list(range(KC)) + list(
                range(KC + 4, NT)
            ):
                finish_tile(i)

            # main loop
            for blk in range(NB):
                avs = [
                    av_pool.tile([P, VW], F32, tag=f"av{t}", name=f"av{t}_{blk}")
                    for t in range(QT)
                ]
                for sk in range(KS):
                    st = ps_pool.tile([P, SC, QB], F32, tag="st", name=f"st{blk}_{sk}")
                    for j in range(SC):
                        for dc in range(DC):
                            nc.tensor.matmul(
                                st[:, j, :],
                                lhsT=kt[:, dc, (sk * SC + j) * P : (sk * SC + j + 1) * P],
                                rhs=qt[:, dc, blk * QB : (blk + 1) * QB],
                                start=(dc == 0),
                                stop=(dc == DC - 1),
                            )
                    pt = pt_pool.tile([P, SC, QB], AV_DT, tag="pt", name=f"pt{blk}_{sk}")
                    nc.scalar.activation(pt, st, Exp)
                    for j in range(SC):
                        kk = sk * SC + j
                        for t in range(QT):
                            nc.tensor.matmul(
                                avs[t],
                                lhsT=pt[:, j, t * P : (t + 1) * P],
                                rhs=va[:, kk, :],
                                start=(kk == 0),
                                stop=(kk == KC - 1),
                            )
                for t in range(QT):
                    rec = small.tile([P, 1], F32, tag="rec")
                    nc.vector.reciprocal(rec, avs[t][:, D : D + 1])
                    ot = out_pool.tile([P, D], F32, tag="ot")
                    nc.vector.tensor_scalar_mul(ot, avs[t][:, :D], rec)
                    row = blk * QB + t * P
                    nc.sync.dma_start(o_d[row : row + P, :], ot)

    nc.compile()
    return nc


_CACHED = {}


def _get_program():
    if "nc" not in _CACHED:
        _CACHED["nc"] = _build_program()
    return _CACHED["nc"]


def _make_in_maps(query, key, value):
    in_maps = []
    for c in range(N_CORES):
        b = c // (N_CORES // B)
        qs = (c % (N_CORES // B)) * NQ
        in_maps.append(
            {
                "q": np.ascontiguousarray(query[b, qs : qs + NQ], dtype=np.float32),
                "k": np.ascontiguousarray(key[b], dtype=np.float32),
                "v": np.ascontiguousarray(value[b], dtype=np.float32),
            }
        )
    return in_maps


def run_sharded(query, key, value, trace=False):
    """Returns (out, BassKernelResults)."""
    nc = _get_program()
    in_maps = _make_in_maps(query, key, value)
    res = run_bass_kernel_spmd(nc, in_maps, core_ids=list(range(N_CORES)), trace=trace)
    out = np.empty((B, NQ_FULL, D), dtype=np.float32)
    for c in range(N_CORES):
        b = c // (N_CORES // B)
        qs = (c % (N_CORES // B)) * NQ
        out[b, qs : qs + NQ] = res.results[c]["o"]
    return out, res


def kernel(query, key, value):
    query = np.asarray(query)
    key = np.asarray(key)
    value = np.asarray(value)
    out, _ = run_sharded(query, key, value)
    return out
